# revision 22
# baseline (speedup 1.0000x reference)
"""Trainium2 Bass kernel for a dense transformer block:
x -> LN1 -> causal MHA (16 heads) -> +residual -> LN2 -> FFN(4x, relu) -> +residual

Full inputs in, full outputs out. Sharding: 8 cores = (batch b in 0..3) x (parity p in 0..1).
Core (b, p) owns query 512-blocks {2j+p : j in 0..1} of batch b (1024 tokens), computes K/V
for the whole batch (duplicated within the pair), runs block-causal attention with a uniform
SPMD program (per-core causal masks passed as data), then proj/LN2/FFN on its own token rows.
No collectives.

Structure (what got it from 990 -> 805 us, then below 805):
 - x2T residual + raw xq ride the Scalar DMA queue at kernel start; group-0 QKV
   weights prefetch behind the xT loads; hq = LN1(xq) is computed directly from
   the xqT input via mu/rstd broadcast-gathered from DRAM with a partition-id
   dynamic offset (replaces a 6MB hT spill+gather bounce).
 - ln_T emits stats for both superchunks before any applies (the in-order DVE
   queue otherwise blocks sc1's stats behind sc0's applies); all LN applies run
   on Vector only (GpSimd shares SBUF ports with DVE and thrashes if both run
   elementwise work concurrently).
 - FFN1 is c-outer so its first matmuls need only the first half of the LN2
   apply; w1 streams twice on the otherwise-idle Scalar queue.
 - The last group's softmax denominator uses a single-lane [1,1024] reciprocal
   (nothing else wants DVE there) to skip two serial DMA hops before proj7.
 - Software-pipelined emission: the per-group attention softmax chain
   (scores -> exp -> mask -> PV) is latency-bound, so group g+1's QKV matmuls and
   group g-1's output-projection partials are emitted as PE "filler" BETWEEN each
   pair's score and PV matmuls. This keeps the in-order PE queue dense, which also
   keeps the HAM clock-gate at 2.4 GHz (a sparse PE stream runs at 1.2 GHz).
 - The output projection is a per-group partial accumulation into the f32 transposed
   residual x2T (PSUM reused via the qkv pool tag), so no separate proj phase exists;
   LN2 stats interleave with the last group's partials.
 - Scores for the two heads are emitted back-to-back as 64-row-tiled matmuls
   (tile_position (0,0)/(64,0)) into different PSUM banks so they can execute
   concurrently; score PSUM tiles pair two key-blocks [128,2,512] and exp/mask run
   as single 1024-wide ops (fewer ACT instruction overheads).
 - V is transposed head-wise with ONE batched DMA XBAR transpose per head per group
   (3D out AP; out offsets must be 16-element aligned -> vx row stride 80).
 - proj/LN2/FFN/output stay in transposed [dmodel-part, token] layout end-to-end:
   no PE transposes, w1/w2 loaded once, bp folded into the residual input host-side,
   output written transposed and un-transposed on the host.
 - ln(x) for own tokens is gathered from the applied hT via a DRAM bounce with a
   partition-id-dependent dynamic DMA offset (no second LayerNorm pass).
 - Softmax denominators ride as a ones-column in the V stationary (row 64), are
   bounced through DRAM per group ([2,1024] -> [128,16]) for a lane-parallel
   reciprocal + Newton step, and applied as one deferred [128,1024] multiply that
   overlaps the next group.
 - LN reciprocals use the same DRAM-bounce trick; LN applies split 6:2 between DVE
   and GpSimd; LN bounce DMAs ride the otherwise-idle GpSimd queue; big loads ride
   Sync; the hT->DRAM spill rides Scalar (keeps Sync head-of-line free).
 - A ~6 us burst of throwaway matmuls at kernel start and dependency-chained dummy
   matmuls through the LN2 tail trip the HAM activity window so LN1 stats and the
   FFN start at full clock.
"""

import numpy as np
import ml_dtypes

BF16 = ml_dtypes.bfloat16

B, T, D = 4, 2048, 1024
H, DH = 16, 64
NG = 8            # head groups of 2 heads
TC = 1024         # tokens per core
QB = 512          # query block
NJ = 2            # local query blocks per core
F4 = 4096
EPS = 1e-5
SCALE = float(D) ** -0.5
N_CORES = 8

_cache = {}


def _build():
    import contextlib
    import concourse.bass as bass
    import concourse.mybir as mybir
    import concourse.tile as tile
    from concourse import bacc

    f32 = mybir.dt.float32
    bf16 = mybir.dt.bfloat16
    AF = mybir.ActivationFunctionType
    ALU = mybir.AluOpType

    nc = bacc.Bacc('TRN2', target_bir_lowering=False, debug=False,
                   num_devices=N_CORES)

    # ---- external I/O (per-core) ----
    xT_d = nc.dram_tensor("xT", [D, T], bf16, kind="ExternalInput")
    xqT_d = nc.dram_tensor("xqT", [D, TC], bf16, kind="ExternalInput")
    xoT_d = nc.dram_tensor("xoT", [D, TC], f32, kind="ExternalInput")
    wq_d = nc.dram_tensor("wqp", [NG, 8, 128, 128], bf16, kind="ExternalInput")
    wk_d = nc.dram_tensor("wkp", [NG, 8, 128, 128], bf16, kind="ExternalInput")
    wv_d = nc.dram_tensor("wvp", [NG, 8, 128, 128], bf16, kind="ExternalInput")
    qb_d = nc.dram_tensor("qbias", [NG, 128], f32, kind="ExternalInput")
    kb_d = nc.dram_tensor("kbias", [NG, 128], f32, kind="ExternalInput")
    vb_d = nc.dram_tensor("vbias", [NG, 128], f32, kind="ExternalInput")
    wp_d = nc.dram_tensor("wpp", [8, 8, 128, 128], bf16, kind="ExternalInput")
    w1_d = nc.dram_tensor("w1p", [32, 8, 128, 128], bf16, kind="ExternalInput")
    w2_d = nc.dram_tensor("w2p", [8, 32, 128, 128], bf16, kind="ExternalInput")
    b1_d = nc.dram_tensor("b1t", [32, 128], f32, kind="ExternalInput")
    b2_d = nc.dram_tensor("b2t", [8, 128], f32, kind="ExternalInput")
    mk_d = nc.dram_tensor("masks", [8, 128, QB], bf16, kind="ExternalInput")
    out_d = nc.dram_tensor("out", [D, TC], f32, kind="ExternalOutput")

    den_dram = nc.dram_tensor("den_scratch", [2, 2, TC], f32)
    rden_dram = nc.dram_tensor("rden_scratch", [2, 2, TC], f32)
    hA_dram = nc.dram_tensor("hA_scratch", [8, 128, T], bf16)
    mu_dram = nc.dram_tensor("mu_scratch", [2, 1024], bf16)
    sd_dram = nc.dram_tensor("sd_scratch", [2, 1024], f32)
    rs_dram = nc.dram_tensor("rs_scratch", [2, 1024], bf16)

    def bcast_ap(dram_ap, parts, n):
        return bass.AP(tensor=dram_ap.tensor, offset=dram_ap.offset,
                       ap=[[0, parts], [1, n]])

    with tile.TileContext(nc) as tc:
        ctx = contextlib.ExitStack()
        with ctx:
            consts = ctx.enter_context(tc.tile_pool(name="consts", bufs=1))
            persist = ctx.enter_context(tc.tile_pool(name="persist", bufs=1))

            # ---------- constants ----------
            ones16 = consts.tile([128, 16], f32)
            nc.vector.memset(ones16, 1.0)
            ones_f = ones16[:, 0:1]
            ones_r = consts.tile([128, 1], bf16)
            nc.vector.tensor_copy(ones_r, ones_f)
            eps_t = consts.tile([1, 1], f32)
            nc.vector.memset(eps_t, EPS)

            def ln_apply(src_all, out_all, sc, mu_b, rstd_b, wpool):
                # all on Vector: GpSimd shares SBUF ports with DVE, so
                # concurrent applies on both engines thrash each other
                for c in range(2):
                    for i in range(8):
                        hsl = bass.ds(sc * 1024 + c * 512, 512)
                        bsl = bass.ds(c * 512, 512)
                        t1 = wpool.tile([128, 512], bf16, tag="t1")
                        nc.vector.tensor_tensor(out=t1,
                                                in0=src_all[:, i, hsl],
                                                in1=mu_b[:, bsl],
                                                op=ALU.subtract)
                        nc.vector.tensor_tensor(out=out_all[:, i, hsl],
                                                in0=t1,
                                                in1=rstd_b[:, bsl],
                                                op=ALU.mult)

            def ln_tail(mus, sqs, src_all, out_all, sc, cb, wpool, spool,
                        pspool, warm_pe=False):
                mu = spool.tile([1, 1024], f32, tag="mu")
                sb = spool.tile([1, 1024], f32, tag="sb")
                for c in range(2):
                    cs = bass.ds(c * 512, 512)
                    nc.scalar.mul(mu[:, cs], mus[c], 1.0 / D)
                    nc.scalar.mul(sb[:, cs], sqs[c], 1.0 / D)
                sc2 = spool.tile([1, 1024], f32, tag="sc2")
                nc.vector.tensor_tensor(out=sc2, in0=mu, in1=mu, op=ALU.mult)
                nc.vector.tensor_tensor(out=sb, in0=sb, in1=sc2,
                                        op=ALU.subtract)
                nc.scalar.activation(out=sb, in_=sb, func=AF.Sqrt, bias=eps_t)
                # tiny dependency-chained matmuls keep the PE activity window
                # non-idle through this serial tail so the next GEMM phase
                # starts at full clock (HAM k=8)
                if warm_pe:
                    warm = pspool.tile([1, 512], f32, tag="warm",
                                       name="warm_a")
                    nc.tensor.matmul(warm, eps_t, sb[:, 0:512], start=True,
                                     stop=True)
                # lane-parallel reciprocal via DRAM bounce
                mub16 = spool.tile([1, 1024], bf16, tag="mub16")
                nc.vector.tensor_copy(mub16, mu)
                nc.gpsimd.dma_start(out=mu_dram[cb, :], in_=mub16)
                nc.gpsimd.dma_start(out=sd_dram[cb, :], in_=sb)
                dd = spool.tile([128, 8], f32, tag="dd")
                nc.gpsimd.dma_start(
                    out=dd, in_=sd_dram[cb, :].rearrange("(p i) -> p i", p=128))
                rr = spool.tile([128, 8], f32, tag="rr")
                nc.vector.reciprocal(rr, dd)
                nt = spool.tile([128, 8], f32, tag="nt")
                nc.vector.tensor_tensor(out=nt, in0=dd, in1=rr, op=ALU.mult)
                nc.vector.tensor_scalar(out=nt, in0=nt, scalar1=-1.0,
                                        scalar2=2.0, op0=ALU.mult, op1=ALU.add)
                if warm_pe:
                    warm2 = pspool.tile([1, 8], f32, tag="warm2",
                                        name="warm_b")
                    nc.tensor.matmul(warm2, ones_f, dd, start=True, stop=True)
                rr16 = spool.tile([128, 8], bf16, tag="rr16")
                nc.vector.tensor_tensor(out=rr16, in0=rr, in1=nt, op=ALU.mult)
                nc.gpsimd.dma_start(
                    out=rs_dram[cb, :].rearrange("(p i) -> p i", p=128),
                    in_=rr16)
                mu_b = wpool.tile([128, 1024], bf16, tag="mu_b")
                nc.gpsimd.dma_start(out=mu_b,
                                    in_=bcast_ap(mu_dram[cb, :], 128, 1024))
                rstd_b = wpool.tile([128, 1024], bf16, tag="rstd_b")
                nc.gpsimd.dma_start(out=rstd_b,
                                    in_=bcast_ap(rs_dram[cb, :], 128, 1024))
                if warm_pe:
                    warm3 = pspool.tile([1, 512], f32, tag="warm",
                                        name="warm_c")
                    nc.tensor.matmul(warm3, ones_r, rstd_b[:, 0:512],
                                     start=True, stop=True)
                return mu_b, rstd_b

            # ---------- transposed layernorm over 1024-token superchunks ----
            # src_all: [128, 8, n_tok] (bf16 in-place) or f32 src with bf16 out
            def ln_T(src_all, n_tok, wpool, spool, pspool, out_all=None):
                # stats for ALL superchunks first, then tails, then applies:
                # keeps sc1's stats matmuls (PE, in-order queue) from waiting
                # behind sc0's DVE apply ops.
                if out_all is None:
                    out_all = src_all
                src_f32 = (src_all.dtype == f32)
                ones_s = ones_f if src_f32 else ones_r
                nsc = n_tok // 1024
                heads = []
                for sc in range(nsc):
                    cb = sc % 2
                    sl = bass.ds(sc * 1024, 1024)
                    mus = [pspool.tile([1, 512], f32, tag=f"mu{c}",
                                       name=f"mu_ps{c}") for c in range(2)]
                    sqs = [pspool.tile([1, 512], f32, tag=f"sq{c}",
                                       name=f"sq_ps{c}") for c in range(2)]
                    for i in range(8):
                        sq = wpool.tile([128, 1024], bf16, tag="sq")
                        if i % 2 == 0:
                            nc.scalar.activation(out=sq, in_=src_all[:, i, sl],
                                                 func=AF.Square)
                        else:
                            nc.vector.tensor_tensor(out=sq,
                                                    in0=src_all[:, i, sl],
                                                    in1=src_all[:, i, sl],
                                                    op=ALU.mult)
                        for c in range(2):
                            cs = bass.ds(sc * 1024 + c * 512, 512)
                            nc.tensor.matmul(mus[c], ones_s, src_all[:, i, cs],
                                             start=(i == 0), stop=(i == 7))
                            nc.tensor.matmul(sqs[c], ones_r,
                                             sq[:, c * 512:(c + 1) * 512],
                                             start=(i == 0), stop=(i == 7))
                    heads.append((sc, ln_tail(mus, sqs, src_all, out_all, sc,
                                              cb, wpool, spool, pspool)))
                for sc, (mu_b, rstd_b) in heads:
                    ln_apply(src_all, out_all, sc, mu_b, rstd_b, wpool)

            # ---------- persistent tiles (live to the end) ----------
            x2T = persist.tile([128, 8, TC], f32, tag="x2T")
            # residual (+bp) preload; proj accumulates into it later.
            # rides the Scalar DMA queue so Sync is free for mask/xT/weights.
            for i in range(8):
                nc.scalar.dma_start(
                    out=x2T[:, i, :], in_=xoT_d[i * 128:(i + 1) * 128, :])

            hctx = contextlib.ExitStack()
            hp = hctx.enter_context(tc.tile_pool(name="h_pool", bufs=1))
            hT = hp.tile([128, 8, T], bf16, tag="hT")
            hq = hp.tile([128, 8, TC], bf16, tag="hq")
            attT = hp.tile([128, 8, TC], bf16, tag="attT")

            attc = hctx.enter_context(tc.tile_pool(name="att_c", bufs=1))
            mask_t = attc.tile([128, 8, QB], bf16)
            nc.sync.dma_start(out=mask_t,
                              in_=mk_d[:, :, :].rearrange("r p q -> p r q"))

            # weight pool opened early so group-0 QKV weights prefetch
            actx = contextlib.ExitStack()
            wgp = actx.enter_context(tc.tile_pool(name="wg_pool", bufs=2))

            def emit_weights(g):
                b = {}
                for nm, dram in (("wqg", wq_d), ("wkg", wk_d),
                                 ("wvg", wv_d)):
                    tl = wgp.tile([128, 8, 128], bf16, tag=nm,
                                  name=f"{nm}{g}")
                    nc.sync.dma_start(out=tl,
                                      in_=dram[g].rearrange("k p c -> p k c"))
                    b[nm] = tl
                return b

            # ---------- LN1 on hT (2048 tokens); hq applied from xqT ------
            with tc.tile_pool(name="ln_work", bufs=3) as lnw, \
                 tc.tile_pool(name="ln_stats", bufs=2) as lns, \
                 tc.tile_pool(name="hq_work", bufs=1) as hqw, \
                 tc.tile_pool(name="ps_ln1", bufs=1, space="PSUM") as ps_ln1:
                # ~3.4us of throwaway matmuls trip the HAM activity window
                # so LN1 stats run at full clock
                for w in range(10):
                    wps = ps_ln1.tile([1, 512], f32, tag="warm",
                                      name=f"warm0_{w}")
                    nc.tensor.matmul(wps, ones_r, mask_t[:, w % 8, :],
                                     start=True, stop=True)
                for c in range(4):
                    for i in range(8):
                        nc.sync.dma_start(
                            out=hT[:, i, c * 512:(c + 1) * 512],
                            in_=xT_d[i * 128:(i + 1) * 128,
                                     c * 512:(c + 1) * 512])
                b_cur = emit_weights(0)
                qbias_t = attc.tile([128, 8], f32)
                nc.sync.dma_start(out=qbias_t,
                                  in_=qb_d[:, :].rearrange("g p -> p g"))
                kbias_t = attc.tile([128, 8], f32)
                nc.sync.dma_start(out=kbias_t,
                                  in_=kb_d[:, :].rearrange("g p -> p g"))
                vbias_t = attc.tile([128, 8], f32)
                nc.sync.dma_start(out=vbias_t,
                                  in_=vb_d[:, :].rearrange("g p -> p g"))
                wp_sb = attc.tile([128, 8, 8, 128], bf16)
                nc.sync.dma_start(
                    out=wp_sb,
                    in_=wp_d[:, :, :, :].rearrange("k o p c -> p k o c"))

                ln_T(hT, T, lnw, lns, ps_ln1)

                # own tokens are 512-blocks {p, p+2}: spill applied hT per
                # superchunk-half (so it starts right after sc0's applies)
                # and gather per j-block with a partition-dependent offset.
                # All on DMA queues — keeps the DVE free for the hT applies.
                qoff = (nc.gpsimd.partition_id() % 2) * 512
                for sc in range(2):
                    ssl = bass.ds(sc * 1024, 1024)
                    for i in range(8):
                        nc.scalar.dma_start(out=hA_dram[i][:, ssl],
                                            in_=hT[:, i, ssl])
                for j in range(2):
                    for i in range(8):
                        hap = hA_dram[i]
                        nc.gpsimd.dma_start(
                            out=hq[:, i, j * 512:(j + 1) * 512],
                            in_=bass.AP(tensor=hap.tensor,
                                        offset=hap.offset + j * 1024 + qoff,
                                        ap=[[T, 128], [1, 512]]))

            # ---------- attention (QKV + scores + PV per group) ----------
            kvp = actx.enter_context(tc.tile_pool(name="kv_pool", bufs=2))
            atw = actx.enter_context(tc.tile_pool(name="att_work", bufs=4))
            dnp = actx.enter_context(tc.tile_pool(name="den_pool", bufs=1))
            with tc.tile_pool(name="ps_qkv", bufs=2, space="PSUM") as ps_qkv, \
                 tc.tile_pool(name="ps_st", bufs=2, space="PSUM") as ps_st, \
                 tc.tile_pool(name="ps_acc", bufs=1, space="PSUM") as ps_acc:
                from collections import deque

                def qkv_events(g, b):
                    ev = []

                    def alloc(b=b, g=g):
                        b["kt"] = kvp.tile([128, T], bf16, tag="kt",
                                           name=f"kt{g}")
                        b["vt"] = kvp.tile([128, T], bf16, tag="vt",
                                           name=f"vt{g}")
                        b["qt"] = kvp.tile([128, TC], bf16, tag="qt",
                                           name=f"qt{g}")
                        b["vxa"] = kvp.tile([128, 16, 80], bf16, tag="vxa",
                                            name=f"vxa{g}")
                        b["vxb"] = kvp.tile([128, 16, 80], bf16, tag="vxb",
                                            name=f"vxb{g}")
                    ev.append(alloc)
                    for n in range(4):
                        def kch(n=n, b=b, g=g):
                            sl = bass.ds(n * 512, 512)
                            ps = ps_qkv.tile([128, 512], f32, tag="qkv",
                                             name=f"psk{g}_{n}")
                            for k in range(8):
                                nc.tensor.matmul(ps, b["wkg"][:, k, :],
                                                 hT[:, k, sl],
                                                 start=(k == 0), stop=(k == 7))
                            nc.vector.tensor_scalar_add(b["kt"][:, sl], ps,
                                                        kbias_t[:, g:g + 1])
                        ev.append(kch)
                    for n in range(4):
                        def vch(n=n, b=b, g=g):
                            sl = bass.ds(n * 512, 512)
                            ps = ps_qkv.tile([128, 512], f32, tag="qkv",
                                             name=f"psv{g}_{n}")
                            for k in range(8):
                                nc.tensor.matmul(ps, b["wvg"][:, k, :],
                                                 hT[:, k, sl],
                                                 start=(k == 0), stop=(k == 7))
                            nc.vector.tensor_scalar_add(b["vt"][:, sl], ps,
                                                        vbias_t[:, g:g + 1])
                        ev.append(vch)

                    def vtr(b=b):
                        nc.vector.tensor_copy(b["vxa"][:, :, 64:65], ones16)
                        nc.vector.tensor_copy(b["vxb"][:, :, 64:65], ones16)
                        nc.sync.dma_start(out=b["vxa"][:, :, 0:64],
                                          in_=b["vt"][0:64, :], transpose=True)
                        nc.sync.dma_start(out=b["vxb"][:, :, 0:64],
                                          in_=b["vt"][64:128, :], transpose=True)
                    ev.append(vtr)
                    for n in range(2):
                        def qch(n=n, b=b, g=g):
                            sl = bass.ds(n * 512, 512)
                            ps = ps_qkv.tile([128, 512], f32, tag="qkv",
                                             name=f"psq{g}_{n}")
                            for k in range(8):
                                nc.tensor.matmul(ps, b["wqg"][:, k, :],
                                                 hq[:, k, sl],
                                                 start=(k == 0), stop=(k == 7))
                            nc.vector.tensor_scalar_add(b["qt"][:, sl], ps,
                                                        qbias_t[:, g:g + 1])
                        ev.append(qch)
                    return ev

                def proj_events(g):
                    ev = []
                    for o in range(8):
                        def pev(o=o, g=g):
                            for c in range(2):
                                cs = bass.ds(c * 512, 512)
                                ps = ps_qkv.tile([128, 512], f32, tag="qkv",
                                                 name=f"pj{g}_{o}_{c}")
                                nc.tensor.matmul(ps, wp_sb[:, g, o, :],
                                                 attT[:, g, cs],
                                                 start=True, stop=True)
                                nc.vector.tensor_tensor(out=x2T[:, o, cs],
                                                        in0=ps,
                                                        in1=x2T[:, o, cs],
                                                        op=ALU.add)
                        ev.append(pev)
                    return ev

                def attention_emit(g, b, fillers):
                    kt, qt = b["kt"], b["qt"]
                    vxa, vxb = b["vxa"], b["vxb"]
                    den0 = dnp.tile([1, TC], f32, tag="den0", name=f"den0_{g}")
                    den1 = dnp.tile([1, TC], f32, tag="den1", name=f"den1_{g}")
                    dds = []
                    pairs_left = 12
                    for j in range(NJ):
                        npair = 4 * j + 4
                        qsl = bass.ds(j * QB, QB)
                        acc0 = ps_acc.tile([65, QB], f32, tag="acc0",
                                           name=f"acc0_{g}_{j}")
                        acc1 = ps_acc.tile([65, QB], f32, tag="acc1",
                                           name=f"acc1_{g}_{j}")
                        for pr in range(npair):
                            st0 = ps_st.tile([128, 2, 512], f32, tag="st",
                                             name=f"st0_{g}_{j}_{pr}")
                            st1 = ps_st.tile([128, 2, 512], f32, tag="st",
                                             name=f"st1_{g}_{j}_{pr}")
                            for t in range(2):
                                kb = pr * 2 + t
                                ksl = bass.ds(kb * 128, 128)
                                nc.tensor.matmul(
                                    st0[:, t, :], kt[0:64, ksl], qt[0:64, qsl],
                                    start=True, stop=True, tile_position=(0, 0))
                                nc.tensor.matmul(
                                    st1[:, t, :], kt[64:128, ksl],
                                    qt[64:128, qsl],
                                    start=True, stop=True,
                                    tile_position=(64, 0))
                            pt0 = atw.tile([128, 2, 512], bf16, tag="pt",
                                           name=f"pt0_{g}_{j}_{pr}")
                            pt1 = atw.tile([128, 2, 512], bf16, tag="pt",
                                           name=f"pt1_{g}_{j}_{pr}")
                            nc.scalar.activation(out=pt0, in_=st0, func=AF.Exp,
                                                 scale=SCALE)
                            nc.scalar.activation(out=pt1, in_=st1, func=AF.Exp,
                                                 scale=SCALE)
                            rel = pr * 2 - 8 * j
                            if rel >= 0:
                                nc.vector.tensor_tensor(
                                    out=pt0, in0=pt0,
                                    in1=mask_t[:, rel:rel + 2, :], op=ALU.mult)
                                nc.vector.tensor_tensor(
                                    out=pt1, in0=pt1,
                                    in1=mask_t[:, rel:rel + 2, :], op=ALU.mult)
                            # PE filler between scores and PV (hides exp+mask)
                            npop = min(len(fillers),
                                       -(-len(fillers) // pairs_left))
                            for _ in range(npop):
                                fillers.popleft()()
                            pairs_left -= 1
                            for t in range(2):
                                kb = pr * 2 + t
                                nc.tensor.matmul(acc0, vxa[:, kb, 0:65],
                                                 pt0[:, t, :],
                                                 start=(kb == 0),
                                                 stop=(kb == 2 * npair - 1))
                                nc.tensor.matmul(acc1, vxb[:, kb, 0:65],
                                                 pt1[:, t, :],
                                                 start=(kb == 0),
                                                 stop=(kb == 2 * npair - 1))
                        nc.vector.tensor_copy(attT[0:64, g, qsl],
                                              acc0[0:64, :])
                        nc.scalar.copy(attT[64:128, g, qsl], acc1[0:64, :])
                        # den copies on Vector (parallel to the attT copy on
                        # Scalar); den DMA stages emitted per-j so j=0's
                        # DRAM round-trip overlaps j=1's score/PV work
                        nc.vector.tensor_copy(den0[:, qsl], acc0[64:65, :])
                        nc.vector.tensor_copy(den1[:, qsl], acc1[64:65, :])
                        gb = g % 2
                        nc.gpsimd.dma_start(out=den_dram[gb, 0, qsl],
                                            in_=den0[:, qsl])
                        nc.gpsimd.dma_start(out=den_dram[gb, 1, qsl],
                                            in_=den1[:, qsl])
                        ddj = dnp.tile([128, 8], f32, tag=f"dd{j}",
                                       name=f"dd{g}_{j}")
                        dap = den_dram[gb]
                        nc.gpsimd.dma_start(
                            out=ddj,
                            in_=bass.AP(tensor=dap.tensor,
                                        offset=dap.offset + j * QB,
                                        ap=[[TC, 2], [8, 64], [1, 8]]))
                        dds.append(ddj)
                    while fillers:
                        fillers.popleft()()
                    # deferred softmax normalization: dd halves already landed
                    # during the group; only the fast lane-parallel recip,
                    # the rden write and the broadcast remain here
                    gb = g % 2
                    rb = dnp.tile([128, TC], f32, tag="rb", name=f"rb{g}")
                    for j, ddj in enumerate(dds):
                        qsl = bass.ds(j * QB, QB)
                        rr = dnp.tile([128, 8], f32, tag=f"rr{j}",
                                      name=f"rr{g}_{j}")
                        nc.vector.reciprocal(rr, ddj)
                        nt2 = dnp.tile([128, 8], f32, tag=f"nt2{j}",
                                       name=f"nt2{g}_{j}")
                        nc.vector.tensor_tensor(out=nt2, in0=ddj, in1=rr,
                                                op=ALU.mult)
                        nc.vector.tensor_scalar(out=nt2, in0=nt2, scalar1=-1.0,
                                                scalar2=2.0, op0=ALU.mult,
                                                op1=ALU.add)
                        nc.vector.tensor_tensor(out=rr, in0=rr, in1=nt2,
                                                op=ALU.mult)
                        rwap = rden_dram[gb]
                        nc.gpsimd.dma_start(
                            out=bass.AP(tensor=rwap.tensor,
                                        offset=rwap.offset + j * QB,
                                        ap=[[TC, 2], [8, 64], [1, 8]]),
                            in_=rr)
                        for hh, ps in ((0, bass.ds(0, 64)),
                                       (1, bass.ds(64, 64))):
                            rap = rden_dram[gb, hh]
                            nc.gpsimd.dma_start(
                                out=rb[ps, qsl],
                                in_=bass.AP(tensor=rap.tensor,
                                            offset=rap.offset + j * QB,
                                            ap=[[0, 64], [1, QB]]))
                        nc.vector.tensor_tensor(out=attT[:, g, qsl],
                                                in0=attT[:, g, qsl],
                                                in1=rb[:, qsl], op=ALU.mult)

                for e in qkv_events(0, b_cur):
                    e()
                for g in range(NG):
                    fillers = deque()
                    if g < NG - 1:
                        b_next = emit_weights(g + 1)
                        fillers.extend(qkv_events(g + 1, b_next))
                    if g >= 1:
                        fillers.extend(proj_events(g - 1))
                    attention_emit(g, b_cur, fillers)
                    if g < NG - 1:
                        b_cur = b_next
            # free attention work pools (keep attT + wp_sb)
            actx.close()

            # ---------- proj(7) with LN2 stats interleaved -> h2T ----------
            h2T = persist.tile([128, 8, TC], bf16, tag="h2T")
            with tc.tile_pool(name="pj_ps", bufs=2, space="PSUM") as ps_pj, \
                 tc.tile_pool(name="ln2_work", bufs=4) as lnw2, \
                 tc.tile_pool(name="ln2_stats", bufs=2) as lns2, \
                 tc.tile_pool(name="ps_ln2", bufs=1, space="PSUM") as ps_ln2:
                mus2 = [ps_ln2.tile([1, 512], f32, tag=f"mu{c}",
                                    name=f"mu2_{c}") for c in range(2)]
                sqs2 = [ps_ln2.tile([1, 512], f32, tag=f"sq{c}",
                                    name=f"sq2_{c}") for c in range(2)]
                for o in range(8):
                    for c in range(2):
                        cs = bass.ds(c * 512, 512)
                        ps = ps_pj.tile([128, 512], f32, tag="pj",
                                        name=f"pj7_{o}_{c}")
                        nc.tensor.matmul(ps, wp_sb[:, NG - 1, o, :],
                                         attT[:, NG - 1, cs],
                                         start=True, stop=True)
                        nc.vector.tensor_tensor(out=x2T[:, o, cs], in0=ps,
                                                in1=x2T[:, o, cs], op=ALU.add)
                    sq = lnw2.tile([128, 1024], bf16, tag="sq")
                    if o % 2 == 0:
                        nc.scalar.activation(out=sq, in_=x2T[:, o, :],
                                             func=AF.Square)
                    else:
                        nc.vector.tensor_tensor(out=sq, in0=x2T[:, o, :],
                                                in1=x2T[:, o, :], op=ALU.mult)
                    for c in range(2):
                        cs = bass.ds(c * 512, 512)
                        nc.tensor.matmul(mus2[c], ones_f, x2T[:, o, cs],
                                         start=(o == 0), stop=(o == 7))
                        nc.tensor.matmul(sqs2[c], ones_r,
                                         sq[:, c * 512:(c + 1) * 512],
                                         start=(o == 0), stop=(o == 7))
                mu_b2, rs_b2 = ln_tail(mus2, sqs2, x2T, h2T, 0, 0, lnw2,
                                       lns2, ps_ln2, warm_pe=True)
                ln_apply(x2T, h2T, 0, mu_b2, rs_b2, lnw2)

            # free hT/hq/attT + attention work pools before FFN allocates uT
            hctx.close()

            # ---------- FFN (transposed throughout) ----------
            with tc.tile_pool(name="ffn_c", bufs=1) as fcc, \
                 tc.tile_pool(name="ffn_w", bufs=3) as fw, \
                 tc.tile_pool(name="ffn_x", bufs=2) as fx, \
                 tc.tile_pool(name="ps_u", bufs=2, space="PSUM") as ps_u, \
                 tc.tile_pool(name="ps_v", bufs=2, space="PSUM") as ps_v:
                b1t = fcc.tile([128, 32], f32)
                nc.sync.dma_start(out=b1t, in_=b1_d[:, :].rearrange("i p -> p i"))
                b2t = fcc.tile([128, 8], f32)
                nc.sync.dma_start(out=b2t, in_=b2_d[:, :].rearrange("o p -> p o"))
                uT = fcc.tile([128, 32, TC], bf16, tag="uT")
                # c-outer: FFN1 starts after only the c=0 half of the LN2
                # apply; w1 streams twice on the otherwise-idle Scalar queue
                for c in range(2):
                    cs = bass.ds(c * 512, 512)
                    for i in range(32):
                        w1t = fw.tile([128, 8, 128], bf16, tag="w1t")
                        nc.scalar.dma_start(
                            out=w1t, in_=w1_d[i].rearrange("k p c -> p k c"))
                        psu = ps_u.tile([128, 512], f32, tag="u")
                        for k in range(8):
                            nc.tensor.matmul(psu, w1t[:, k, :],
                                             h2T[:, k, cs],
                                             start=(k == 0), stop=(k == 7))
                        nc.scalar.activation(out=uT[:, i, cs], in_=psu,
                                             func=AF.Relu,
                                             bias=b1t[:, i:i + 1])
                for o in range(8):
                    w2t = fw.tile([128, 32, 128], bf16, tag="w2t")
                    nc.sync.dma_start(
                        out=w2t, in_=w2_d[o].rearrange("i p c -> p i c"))
                    pso = ps_v.tile([128, 2, 512], f32, tag="v")
                    for c in range(2):
                        cs = bass.ds(c * 512, 512)
                        for i in range(32):
                            nc.tensor.matmul(pso[:, c, :], w2t[:, i, :],
                                             uT[:, i, cs],
                                             start=(i == 0), stop=(i == 31))
                    ot = fx.tile([128, 1024], f32, tag="ot")
                    nc.vector.tensor_scalar_add(ot, pso, b2t[:, o:o + 1])
                    nc.vector.tensor_tensor(out=ot, in0=ot, in1=x2T[:, o, :],
                                            op=ALU.add)
                    nc.sync.dma_start(out=out_d[o * 128:(o + 1) * 128, :], in_=ot)

    nc.compile()
    return nc


def _prep_shared(wq, wk, wv, wp, bp, w1, b1, w2, b2, g1, be1, g2, be2):
    c = np.ascontiguousarray
    f = np.float32

    def cf(a):
        return c(np.asarray(a, f))

    g1 = cf(g1)
    be1 = cf(be1)
    g2 = cf(g2)
    be2 = cf(be2)
    wq0, wk0, wv0, w10 = cf(wq), cf(wk), cf(wv), cf(w1)
    # fold LN gains into weights; LN biases become post-projection biases
    wqs = wq0 * g1[:, None]
    wks = wk0 * g1[:, None]
    wvs = wv0 * g1[:, None]
    w1s = w10 * g2[:, None]
    qbias = (wq0.T @ be1).astype(f)
    kbias = (wk0.T @ be1).astype(f)
    vbias = (wv0.T @ be1).astype(f)
    b1n = (cf(b1) + w10.T @ be2).astype(f)
    return {
        "wqp": c(wqs.reshape(8, 128, 8, 128).transpose(2, 0, 1, 3)).astype(BF16),
        "wkp": c(wks.reshape(8, 128, 8, 128).transpose(2, 0, 1, 3)).astype(BF16),
        "wvp": c(wvs.reshape(8, 128, 8, 128).transpose(2, 0, 1, 3)).astype(BF16),
        "qbias": c(qbias.reshape(NG, 128)),
        "kbias": c(kbias.reshape(NG, 128)),
        "vbias": c(vbias.reshape(NG, 128)),
        "wpp": c(cf(wp).reshape(8, 128, 8, 128).transpose(0, 2, 1, 3)).astype(BF16),
        "w1p": c(w1s.reshape(8, 128, 32, 128).transpose(2, 0, 1, 3)).astype(BF16),
        "w2p": c(cf(w2).reshape(32, 128, 8, 128).transpose(2, 0, 1, 3)).astype(BF16),
        "b1t": c(b1n.reshape(32, 128)),
        "b2t": c(cf(b2).reshape(8, 128)),
        "bp": cf(bp),
    }


def _own_idx(p):
    return (np.arange(NJ)[:, None] * 1024 + p * QB + np.arange(QB)[None, :]).ravel()


def _masks(p):
    m = np.zeros((8, 128, QB), np.float32)
    k = np.arange(128)[:, None]
    q = np.arange(QB)[None, :]
    for rel in range(8):
        m[rel] = (128 * rel + k <= QB * p + q).astype(np.float32)
    return m


def _make_in_maps(x, shared):
    in_maps = []
    bp = shared["bp"]
    for cid in range(N_CORES):
        b, p = cid // 2, cid % 2
        xb = np.asarray(x[b], np.float32)
        idx = _own_idx(p)
        xq = xb[idx]
        m = {k: v for k, v in shared.items() if k != "bp"}
        m["xT"] = np.ascontiguousarray(xb.T).astype(BF16)
        m["xqT"] = np.ascontiguousarray(xq.T).astype(BF16)
        m["xoT"] = np.ascontiguousarray((xq + bp[None, :]).T.astype(np.float32))
        m["masks"] = _masks(p).astype(BF16)
        in_maps.append(m)
    return in_maps


def kernel(**inputs):
    from concourse.bass_utils import run_bass_kernel_spmd

    if "nc" not in _cache:
        _cache["nc"] = _build()
    nc = _cache["nc"]

    shared = _prep_shared(
        inputs["wq"], inputs["wk"], inputs["wv"], inputs["wp"], inputs["bp"],
        inputs["w1"], inputs["b1"], inputs["w2"], inputs["b2"],
        inputs["g1"], inputs["be1"], inputs["g2"], inputs["be2"])
    in_maps = _make_in_maps(inputs["x"], shared)

    res = run_bass_kernel_spmd(nc, in_maps, list(range(N_CORES)))
    out = np.empty((B, T, D), np.float32)
    for cid in range(N_CORES):
        b, p = cid // 2, cid % 2
        out[b][_own_idx(p)] = res.results[cid]["out"].T
    return out



# revision 24
# speedup vs baseline: 1.0577x; 1.0577x over previous
"""Trainium2 Bass kernel for a dense transformer block:
x -> LN1 -> causal MHA (16 heads) -> +residual -> LN2 -> FFN(4x, relu) -> +residual

Full inputs in, full outputs out. Sharding: 8 cores = (batch b in 0..3) x (parity p in 0..1).
Core (b, p) owns query 512-blocks {2j+p : j in 0..1} of batch b (1024 tokens), computes K/V
for the whole batch (duplicated within the pair), runs block-causal attention with a uniform
SPMD program (per-core causal masks passed as data), then proj/LN2/FFN on its own token rows.
No collectives.

Structure (what got it from 990 -> 805 us, then below 805):
 - x2T residual + raw xq ride the Scalar DMA queue at kernel start; group-0 QKV
   weights prefetch behind the xT loads; hq = LN1(xq) is computed directly from
   the xqT input via mu/rstd broadcast-gathered from DRAM with a partition-id
   dynamic offset (replaces a 6MB hT spill+gather bounce).
 - ln_T emits stats for both superchunks before any applies (the in-order DVE
   queue otherwise blocks sc1's stats behind sc0's applies); all LN applies run
   on Vector only (GpSimd shares SBUF ports with DVE and thrashes if both run
   elementwise work concurrently).
 - FFN1 is c-outer so its first matmuls need only the first half of the LN2
   apply; w1 streams twice on the otherwise-idle Scalar queue.
 - The last group's softmax denominator uses a single-lane [1,1024] reciprocal
   (nothing else wants DVE there) to skip two serial DMA hops before proj7.
 - Software-pipelined emission: the per-group attention softmax chain
   (scores -> exp -> mask -> PV) is latency-bound, so group g+1's QKV matmuls and
   group g-1's output-projection partials are emitted as PE "filler" BETWEEN each
   pair's score and PV matmuls. This keeps the in-order PE queue dense, which also
   keeps the HAM clock-gate at 2.4 GHz (a sparse PE stream runs at 1.2 GHz).
 - The output projection is a per-group partial accumulation into the f32 transposed
   residual x2T (PSUM reused via the qkv pool tag), so no separate proj phase exists;
   LN2 stats interleave with the last group's partials.
 - Scores for the two heads are emitted back-to-back as 64-row-tiled matmuls
   (tile_position (0,0)/(64,0)) into different PSUM banks so they can execute
   concurrently; score PSUM tiles pair two key-blocks [128,2,512] and exp/mask run
   as single 1024-wide ops (fewer ACT instruction overheads).
 - V is transposed head-wise with ONE batched DMA XBAR transpose per head per group
   (3D out AP; out offsets must be 16-element aligned -> vx row stride 80).
 - proj/LN2/FFN/output stay in transposed [dmodel-part, token] layout end-to-end:
   no PE transposes, w1/w2 loaded once, bp folded into the residual input host-side,
   output written transposed and un-transposed on the host.
 - ln(x) for own tokens is gathered from the applied hT via a DRAM bounce with a
   partition-id-dependent dynamic DMA offset (no second LayerNorm pass).
 - Softmax denominators ride as a ones-column in the V stationary (row 64), are
   bounced through DRAM per group ([2,1024] -> [128,16]) for a lane-parallel
   reciprocal + Newton step, and applied as one deferred [128,1024] multiply that
   overlaps the next group.
 - LN reciprocals use the same DRAM-bounce trick; LN applies split 6:2 between DVE
   and GpSimd; LN bounce DMAs ride the otherwise-idle GpSimd queue; big loads ride
   Sync; the hT->DRAM spill rides Scalar (keeps Sync head-of-line free).
 - A ~6 us burst of throwaway matmuls at kernel start and dependency-chained dummy
   matmuls through the LN2 tail trip the HAM activity window so LN1 stats and the
   FFN start at full clock.
"""

import numpy as np
import ml_dtypes

BF16 = ml_dtypes.bfloat16

B, T, D = 4, 2048, 1024
H, DH = 16, 64
NG = 8            # head groups of 2 heads
TC = 1024         # tokens per core
QB = 512          # query block
NJ = 2            # local query blocks per core
F4 = 4096
EPS = 1e-5
SCALE = float(D) ** -0.5
N_CORES = 8

_cache = {}


def _build():
    import contextlib
    import concourse.bass as bass
    import concourse.mybir as mybir
    import concourse.tile as tile
    from concourse import bacc

    f32 = mybir.dt.float32
    bf16 = mybir.dt.bfloat16
    AF = mybir.ActivationFunctionType
    ALU = mybir.AluOpType

    nc = bacc.Bacc('TRN2', target_bir_lowering=False, debug=False,
                   num_devices=N_CORES)

    # ---- external I/O (per-core) ----
    xT_d = nc.dram_tensor("xT", [D, T], bf16, kind="ExternalInput")
    xqT_d = nc.dram_tensor("xqT", [D, TC], bf16, kind="ExternalInput")
    xoT_d = nc.dram_tensor("xoT", [D, TC], f32, kind="ExternalInput")
    wq_d = nc.dram_tensor("wqp", [NG, 8, 128, 128], bf16, kind="ExternalInput")
    wk_d = nc.dram_tensor("wkp", [NG, 8, 128, 128], bf16, kind="ExternalInput")
    wv_d = nc.dram_tensor("wvp", [NG, 8, 128, 128], bf16, kind="ExternalInput")
    qb_d = nc.dram_tensor("qbias", [NG, 128], f32, kind="ExternalInput")
    kb_d = nc.dram_tensor("kbias", [NG, 128], f32, kind="ExternalInput")
    vb_d = nc.dram_tensor("vbias", [NG, 128], f32, kind="ExternalInput")
    wp_d = nc.dram_tensor("wpp", [8, 8, 128, 128], bf16, kind="ExternalInput")
    w1_d = nc.dram_tensor("w1p", [32, 8, 128, 128], bf16, kind="ExternalInput")
    w2_d = nc.dram_tensor("w2p", [8, 32, 128, 128], bf16, kind="ExternalInput")
    b1_d = nc.dram_tensor("b1t", [32, 128], f32, kind="ExternalInput")
    b2_d = nc.dram_tensor("b2t", [8, 128], f32, kind="ExternalInput")
    mk_d = nc.dram_tensor("masks", [8, 128, QB], bf16, kind="ExternalInput")
    out_d = nc.dram_tensor("out", [D, TC], f32, kind="ExternalOutput")

    den_dram = nc.dram_tensor("den_scratch", [2, 2, TC], f32)
    rden_dram = nc.dram_tensor("rden_scratch", [2, 2, TC], f32)
    hA_dram = nc.dram_tensor("hA_scratch", [8, 128, T], bf16)
    mu_dram = nc.dram_tensor("mu_scratch", [2, 1024], bf16)
    sd_dram = nc.dram_tensor("sd_scratch", [2, 1024], f32)
    rs_dram = nc.dram_tensor("rs_scratch", [2, 1024], bf16)

    def bcast_ap(dram_ap, parts, n):
        return bass.AP(tensor=dram_ap.tensor, offset=dram_ap.offset,
                       ap=[[0, parts], [1, n]])

    with tile.TileContext(nc) as tc:
        ctx = contextlib.ExitStack()
        with ctx:
            consts = ctx.enter_context(tc.tile_pool(name="consts", bufs=1))
            persist = ctx.enter_context(tc.tile_pool(name="persist", bufs=1))

            # ---------- constants ----------
            ones16 = consts.tile([128, 16], f32)
            nc.vector.memset(ones16, 1.0)
            ones_f = ones16[:, 0:1]
            ones_r = consts.tile([128, 1], bf16)
            nc.vector.tensor_copy(ones_r, ones_f)
            eps_t = consts.tile([1, 1], f32)
            nc.vector.memset(eps_t, EPS)

            def ln_apply(src_all, out_all, sc, mu_b, rstd_b, wpool):
                # all on Vector: GpSimd shares SBUF ports with DVE, so
                # concurrent applies on both engines thrash each other
                for c in range(2):
                    for i in range(8):
                        hsl = bass.ds(sc * 1024 + c * 512, 512)
                        bsl = bass.ds(c * 512, 512)
                        t1 = wpool.tile([128, 512], bf16, tag="t1")
                        nc.vector.tensor_tensor(out=t1,
                                                in0=src_all[:, i, hsl],
                                                in1=mu_b[:, bsl],
                                                op=ALU.subtract)
                        nc.vector.tensor_tensor(out=out_all[:, i, hsl],
                                                in0=t1,
                                                in1=rstd_b[:, bsl],
                                                op=ALU.mult)

            def ln_tail(mus, sqs, src_all, out_all, sc, cb, wpool, spool,
                        pspool, warm_pe=False):
                mu = spool.tile([1, 1024], f32, tag="mu")
                sb = spool.tile([1, 1024], f32, tag="sb")
                for c in range(2):
                    cs = bass.ds(c * 512, 512)
                    nc.scalar.mul(mu[:, cs], mus[c], 1.0 / D)
                    nc.scalar.mul(sb[:, cs], sqs[c], 1.0 / D)
                sc2 = spool.tile([1, 1024], f32, tag="sc2")
                nc.vector.tensor_tensor(out=sc2, in0=mu, in1=mu, op=ALU.mult)
                nc.vector.tensor_tensor(out=sb, in0=sb, in1=sc2,
                                        op=ALU.subtract)
                nc.scalar.activation(out=sb, in_=sb, func=AF.Sqrt, bias=eps_t)
                # tiny dependency-chained matmuls keep the PE activity window
                # non-idle through this serial tail so the next GEMM phase
                # starts at full clock (HAM k=8)
                if warm_pe:
                    warm = pspool.tile([1, 512], f32, tag="warm",
                                       name="warm_a")
                    nc.tensor.matmul(warm, eps_t, sb[:, 0:512], start=True,
                                     stop=True)
                # lane-parallel reciprocal via DRAM bounce
                mub16 = spool.tile([1, 1024], bf16, tag="mub16")
                nc.vector.tensor_copy(mub16, mu)
                nc.gpsimd.dma_start(out=mu_dram[cb, :], in_=mub16)
                nc.gpsimd.dma_start(out=sd_dram[cb, :], in_=sb)
                dd = spool.tile([128, 8], f32, tag="dd")
                nc.gpsimd.dma_start(
                    out=dd, in_=sd_dram[cb, :].rearrange("(p i) -> p i", p=128))
                rr = spool.tile([128, 8], f32, tag="rr")
                nc.vector.reciprocal(rr, dd)
                nt = spool.tile([128, 8], f32, tag="nt")
                nc.vector.tensor_tensor(out=nt, in0=dd, in1=rr, op=ALU.mult)
                nc.vector.tensor_scalar(out=nt, in0=nt, scalar1=-1.0,
                                        scalar2=2.0, op0=ALU.mult, op1=ALU.add)
                if warm_pe:
                    warm2 = pspool.tile([1, 8], f32, tag="warm2",
                                        name="warm_b")
                    nc.tensor.matmul(warm2, ones_f, dd, start=True, stop=True)
                rr16 = spool.tile([128, 8], bf16, tag="rr16")
                nc.vector.tensor_tensor(out=rr16, in0=rr, in1=nt, op=ALU.mult)
                nc.gpsimd.dma_start(
                    out=rs_dram[cb, :].rearrange("(p i) -> p i", p=128),
                    in_=rr16)
                mu_b = wpool.tile([128, 1024], bf16, tag="mu_b")
                nc.gpsimd.dma_start(out=mu_b,
                                    in_=bcast_ap(mu_dram[cb, :], 128, 1024))
                rstd_b = wpool.tile([128, 1024], bf16, tag="rstd_b")
                nc.gpsimd.dma_start(out=rstd_b,
                                    in_=bcast_ap(rs_dram[cb, :], 128, 1024))
                if warm_pe:
                    warm3 = pspool.tile([1, 512], f32, tag="warm",
                                        name="warm_c")
                    nc.tensor.matmul(warm3, ones_r, rstd_b[:, 0:512],
                                     start=True, stop=True)
                return mu_b, rstd_b

            # ---------- transposed layernorm over 1024-token superchunks ----
            # src_all: [128, 8, n_tok] (bf16 in-place) or f32 src with bf16 out
            def ln_T(src_all, n_tok, wpool, spool, pspool, out_all=None):
                # stats for ALL superchunks first, then tails, then applies:
                # keeps sc1's stats matmuls (PE, in-order queue) from waiting
                # behind sc0's DVE apply ops.
                if out_all is None:
                    out_all = src_all
                src_f32 = (src_all.dtype == f32)
                ones_s = ones_f if src_f32 else ones_r
                nsc = n_tok // 1024
                heads = []
                for sc in range(nsc):
                    cb = sc % 2
                    sl = bass.ds(sc * 1024, 1024)
                    mus = [pspool.tile([1, 512], f32, tag=f"mu{c}",
                                       name=f"mu_ps{c}") for c in range(2)]
                    sqs = [pspool.tile([1, 512], f32, tag=f"sq{c}",
                                       name=f"sq_ps{c}") for c in range(2)]
                    for i in range(8):
                        sq = wpool.tile([128, 1024], bf16, tag="sq")
                        if i % 2 == 0:
                            nc.scalar.activation(out=sq, in_=src_all[:, i, sl],
                                                 func=AF.Square)
                        else:
                            nc.vector.tensor_tensor(out=sq,
                                                    in0=src_all[:, i, sl],
                                                    in1=src_all[:, i, sl],
                                                    op=ALU.mult)
                        for c in range(2):
                            cs = bass.ds(sc * 1024 + c * 512, 512)
                            nc.tensor.matmul(mus[c], ones_s, src_all[:, i, cs],
                                             start=(i == 0), stop=(i == 7))
                            nc.tensor.matmul(sqs[c], ones_r,
                                             sq[:, c * 512:(c + 1) * 512],
                                             start=(i == 0), stop=(i == 7))
                    heads.append((sc, ln_tail(mus, sqs, src_all, out_all, sc,
                                              cb, wpool, spool, pspool)))
                for sc, (mu_b, rstd_b) in heads:
                    ln_apply(src_all, out_all, sc, mu_b, rstd_b, wpool)

            # ---------- persistent tiles (live to the end) ----------
            x2T = persist.tile([128, 8, TC], f32, tag="x2T")
            # residual (+bp) preload; proj accumulates into it later.
            # rides the Scalar DMA queue so Sync is free for mask/xT/weights.
            for i in range(8):
                nc.scalar.dma_start(
                    out=x2T[:, i, :], in_=xoT_d[i * 128:(i + 1) * 128, :])

            hctx = contextlib.ExitStack()
            hp = hctx.enter_context(tc.tile_pool(name="h_pool", bufs=1))
            hT = hp.tile([128, 8, T], bf16, tag="hT")
            hq = hp.tile([128, 8, TC], bf16, tag="hq")
            attT = hp.tile([128, 8, TC], bf16, tag="attT")

            attc = hctx.enter_context(tc.tile_pool(name="att_c", bufs=1))
            mask_t = attc.tile([128, 8, QB], bf16)
            nc.sync.dma_start(out=mask_t,
                              in_=mk_d[:, :, :].rearrange("r p q -> p r q"))

            # weight pool opened early so group-0 QKV weights prefetch
            actx = contextlib.ExitStack()
            wgp = actx.enter_context(tc.tile_pool(name="wg_pool", bufs=2))

            def emit_weights(g):
                b = {}
                for nm, dram in (("wqg", wq_d), ("wkg", wk_d),
                                 ("wvg", wv_d)):
                    tl = wgp.tile([128, 8, 128], bf16, tag=nm,
                                  name=f"{nm}{g}")
                    nc.sync.dma_start(out=tl,
                                      in_=dram[g].rearrange("k p c -> p k c"))
                    b[nm] = tl
                return b

            # ---------- LN1 on hT (2048 tokens); hq applied from xqT ------
            with tc.tile_pool(name="ln_work", bufs=3) as lnw, \
                 tc.tile_pool(name="ln_stats", bufs=2) as lns, \
                 tc.tile_pool(name="hq_work", bufs=1) as hqw, \
                 tc.tile_pool(name="ps_ln1", bufs=1, space="PSUM") as ps_ln1:
                # ~3.4us of throwaway matmuls trip the HAM activity window
                # so LN1 stats run at full clock
                for w in range(10):
                    wps = ps_ln1.tile([1, 512], f32, tag="warm",
                                      name=f"warm0_{w}")
                    nc.tensor.matmul(wps, ones_r, mask_t[:, w % 8, :],
                                     start=True, stop=True)
                for c in range(4):
                    for i in range(8):
                        nc.sync.dma_start(
                            out=hT[:, i, c * 512:(c + 1) * 512],
                            in_=xT_d[i * 128:(i + 1) * 128,
                                     c * 512:(c + 1) * 512])
                b_cur = emit_weights(0)
                qbias_t = attc.tile([128, 8], f32)
                nc.sync.dma_start(out=qbias_t,
                                  in_=qb_d[:, :].rearrange("g p -> p g"))
                kbias_t = attc.tile([128, 8], f32)
                nc.sync.dma_start(out=kbias_t,
                                  in_=kb_d[:, :].rearrange("g p -> p g"))
                vbias_t = attc.tile([128, 8], f32)
                nc.sync.dma_start(out=vbias_t,
                                  in_=vb_d[:, :].rearrange("g p -> p g"))
                wp_sb = attc.tile([128, 8, 8, 128], bf16)
                nc.sync.dma_start(
                    out=wp_sb,
                    in_=wp_d[:, :, :, :].rearrange("k o p c -> p k o c"))

                ln_T(hT, T, lnw, lns, ps_ln1)

                # own tokens are 512-blocks {p, p+2}: spill applied hT per
                # superchunk-half (so it starts right after sc0's applies)
                # and gather per j-block with a partition-dependent offset.
                # All on DMA queues — keeps the DVE free for the hT applies.
                qoff = (nc.gpsimd.partition_id() % 2) * 512
                for sc in range(2):
                    ssl = bass.ds(sc * 1024, 1024)
                    for i in range(8):
                        nc.scalar.dma_start(out=hA_dram[i][:, ssl],
                                            in_=hT[:, i, ssl])
                for j in range(2):
                    for i in range(8):
                        hap = hA_dram[i]
                        nc.gpsimd.dma_start(
                            out=hq[:, i, j * 512:(j + 1) * 512],
                            in_=bass.AP(tensor=hap.tensor,
                                        offset=hap.offset + j * 1024 + qoff,
                                        ap=[[T, 128], [1, 512]]))

            # ---------- attention (QKV + scores + PV per group) ----------
            kvp = actx.enter_context(tc.tile_pool(name="kv_pool", bufs=2))
            atw = actx.enter_context(tc.tile_pool(name="att_work", bufs=4))
            dnp = actx.enter_context(tc.tile_pool(name="den_pool", bufs=1))
            with tc.tile_pool(name="ps_qkv", bufs=2, space="PSUM") as ps_qkv, \
                 tc.tile_pool(name="ps_st", bufs=2, space="PSUM") as ps_st, \
                 tc.tile_pool(name="ps_acc", bufs=1, space="PSUM") as ps_acc:
                from collections import deque

                def qkv_events(g, b):
                    ev = []

                    def alloc(b=b, g=g):
                        b["kt"] = kvp.tile([128, T], bf16, tag="kt",
                                           name=f"kt{g}")
                        b["vt"] = kvp.tile([128, T], bf16, tag="vt",
                                           name=f"vt{g}")
                        b["qt"] = kvp.tile([128, TC], bf16, tag="qt",
                                           name=f"qt{g}")
                        b["vxa"] = kvp.tile([128, 16, 80], bf16, tag="vxa",
                                            name=f"vxa{g}")
                        b["vxb"] = kvp.tile([128, 16, 80], bf16, tag="vxb",
                                            name=f"vxb{g}")
                    ev.append(alloc)
                    for n in range(4):
                        def kch(n=n, b=b, g=g):
                            sl = bass.ds(n * 512, 512)
                            ps = ps_qkv.tile([128, 512], f32, tag="qkv",
                                             name=f"psk{g}_{n}")
                            for k in range(8):
                                nc.tensor.matmul(ps, b["wkg"][:, k, :],
                                                 hT[:, k, sl],
                                                 start=(k == 0), stop=(k == 7))
                            nc.vector.tensor_scalar_add(b["kt"][:, sl], ps,
                                                        kbias_t[:, g:g + 1])
                        ev.append(kch)
                    for n in range(4):
                        def vch(n=n, b=b, g=g):
                            sl = bass.ds(n * 512, 512)
                            ps = ps_qkv.tile([128, 512], f32, tag="qkv",
                                             name=f"psv{g}_{n}")
                            for k in range(8):
                                nc.tensor.matmul(ps, b["wvg"][:, k, :],
                                                 hT[:, k, sl],
                                                 start=(k == 0), stop=(k == 7))
                            nc.vector.tensor_scalar_add(b["vt"][:, sl], ps,
                                                        vbias_t[:, g:g + 1])
                        ev.append(vch)

                    def vtr(b=b):
                        nc.vector.tensor_copy(b["vxa"][:, :, 64:65], ones16)
                        nc.vector.tensor_copy(b["vxb"][:, :, 64:65], ones16)
                        nc.sync.dma_start(out=b["vxa"][:, :, 0:64],
                                          in_=b["vt"][0:64, :], transpose=True)
                        nc.sync.dma_start(out=b["vxb"][:, :, 0:64],
                                          in_=b["vt"][64:128, :], transpose=True)
                    ev.append(vtr)
                    for n in range(2):
                        def qch(n=n, b=b, g=g):
                            sl = bass.ds(n * 512, 512)
                            ps = ps_qkv.tile([128, 512], f32, tag="qkv",
                                             name=f"psq{g}_{n}")
                            for k in range(8):
                                nc.tensor.matmul(ps, b["wqg"][:, k, :],
                                                 hq[:, k, sl],
                                                 start=(k == 0), stop=(k == 7))
                            nc.vector.tensor_scalar_add(b["qt"][:, sl], ps,
                                                        qbias_t[:, g:g + 1])
                        ev.append(qch)
                    return ev

                def proj_events(g):
                    ev = []
                    for o in range(8):
                        def pev(o=o, g=g):
                            for c in range(2):
                                cs = bass.ds(c * 512, 512)
                                ps = ps_qkv.tile([128, 512], f32, tag="qkv",
                                                 name=f"pj{g}_{o}_{c}")
                                nc.tensor.matmul(ps, wp_sb[:, g, o, :],
                                                 attT[:, g, cs],
                                                 start=True, stop=True)
                                nc.vector.tensor_tensor(out=x2T[:, o, cs],
                                                        in0=ps,
                                                        in1=x2T[:, o, cs],
                                                        op=ALU.add)
                        ev.append(pev)
                    return ev

                def attention_emit(g, b, fillers):
                    kt, qt = b["kt"], b["qt"]
                    vxa, vxb = b["vxa"], b["vxb"]
                    den0 = dnp.tile([1, TC], f32, tag="den0", name=f"den0_{g}")
                    den1 = dnp.tile([1, TC], f32, tag="den1", name=f"den1_{g}")
                    pairs_left = 12
                    for j in range(NJ):
                        npair = 4 * j + 4
                        qsl = bass.ds(j * QB, QB)
                        acc0 = ps_acc.tile([65, QB], f32, tag="acc0",
                                           name=f"acc0_{g}_{j}")
                        acc1 = ps_acc.tile([65, QB], f32, tag="acc1",
                                           name=f"acc1_{g}_{j}")
                        for pr in range(npair):
                            st0 = ps_st.tile([128, 2, 512], f32, tag="st",
                                             name=f"st0_{g}_{j}_{pr}")
                            st1 = ps_st.tile([128, 2, 512], f32, tag="st",
                                             name=f"st1_{g}_{j}_{pr}")
                            for t in range(2):
                                kb = pr * 2 + t
                                ksl = bass.ds(kb * 128, 128)
                                nc.tensor.matmul(
                                    st0[:, t, :], kt[0:64, ksl], qt[0:64, qsl],
                                    start=True, stop=True, tile_position=(0, 0))
                                nc.tensor.matmul(
                                    st1[:, t, :], kt[64:128, ksl],
                                    qt[64:128, qsl],
                                    start=True, stop=True,
                                    tile_position=(64, 0))
                            pt0 = atw.tile([128, 2, 512], bf16, tag="pt",
                                           name=f"pt0_{g}_{j}_{pr}")
                            pt1 = atw.tile([128, 2, 512], bf16, tag="pt",
                                           name=f"pt1_{g}_{j}_{pr}")
                            nc.scalar.activation(out=pt0, in_=st0, func=AF.Exp,
                                                 scale=SCALE)
                            nc.scalar.activation(out=pt1, in_=st1, func=AF.Exp,
                                                 scale=SCALE)
                            rel = pr * 2 - 8 * j
                            if rel >= 0:
                                nc.vector.tensor_tensor(
                                    out=pt0, in0=pt0,
                                    in1=mask_t[:, rel:rel + 2, :], op=ALU.mult)
                                nc.vector.tensor_tensor(
                                    out=pt1, in0=pt1,
                                    in1=mask_t[:, rel:rel + 2, :], op=ALU.mult)
                            # PE filler between scores and PV (hides exp+mask)
                            npop = min(len(fillers),
                                       -(-len(fillers) // pairs_left))
                            for _ in range(npop):
                                fillers.popleft()()
                            pairs_left -= 1
                            for t in range(2):
                                kb = pr * 2 + t
                                nc.tensor.matmul(acc0, vxa[:, kb, 0:65],
                                                 pt0[:, t, :],
                                                 start=(kb == 0),
                                                 stop=(kb == 2 * npair - 1))
                                nc.tensor.matmul(acc1, vxb[:, kb, 0:65],
                                                 pt1[:, t, :],
                                                 start=(kb == 0),
                                                 stop=(kb == 2 * npair - 1))
                        nc.vector.tensor_copy(attT[0:64, g, qsl],
                                              acc0[0:64, :])
                        nc.scalar.copy(attT[64:128, g, qsl], acc1[0:64, :])
                        nc.scalar.copy(den0[:, qsl], acc0[64:65, :])
                        nc.scalar.copy(den1[:, qsl], acc1[64:65, :])
                    while fillers:
                        fillers.popleft()()
                    # deferred softmax normalization (overlaps next group)
                    gb = g % 2
                    nc.gpsimd.dma_start(out=den_dram[gb, 0], in_=den0)
                    nc.gpsimd.dma_start(out=den_dram[gb, 1], in_=den1)
                    dd = dnp.tile([128, 16], f32, tag="dd", name=f"dd{g}")
                    nc.gpsimd.dma_start(
                        out=dd,
                        in_=den_dram[gb].rearrange("h (p i) -> (h p) i", p=64))
                    rr = dnp.tile([128, 16], f32, tag="rr", name=f"rr{g}")
                    nc.vector.reciprocal(rr, dd)
                    nt2 = dnp.tile([128, 16], f32, tag="nt2", name=f"nt2{g}")
                    nc.vector.tensor_tensor(out=nt2, in0=dd, in1=rr,
                                            op=ALU.mult)
                    nc.vector.tensor_scalar(out=nt2, in0=nt2, scalar1=-1.0,
                                            scalar2=2.0, op0=ALU.mult,
                                            op1=ALU.add)
                    nc.vector.tensor_tensor(out=rr, in0=rr, in1=nt2,
                                            op=ALU.mult)
                    nc.gpsimd.dma_start(
                        out=rden_dram[gb].rearrange("h (p i) -> (h p) i", p=64),
                        in_=rr)
                    rb = dnp.tile([128, TC], f32, tag="rb", name=f"rb{g}")
                    nc.gpsimd.dma_start(out=rb[0:64, :],
                                        in_=bcast_ap(rden_dram[gb, 0, :], 64,
                                                     TC))
                    nc.gpsimd.dma_start(out=rb[64:128, :],
                                        in_=bcast_ap(rden_dram[gb, 1, :], 64,
                                                     TC))
                    nc.vector.tensor_tensor(out=attT[:, g, :],
                                            in0=attT[:, g, :],
                                            in1=rb, op=ALU.mult)

                for e in qkv_events(0, b_cur):
                    e()
                for g in range(NG):
                    fillers = deque()
                    if g < NG - 1:
                        b_next = emit_weights(g + 1)
                        fillers.extend(qkv_events(g + 1, b_next))
                    if g >= 1:
                        fillers.extend(proj_events(g - 1))
                    attention_emit(g, b_cur, fillers)
                    if g < NG - 1:
                        b_cur = b_next
            # free attention work pools (keep attT + wp_sb)
            actx.close()

            # ---------- proj(7) with LN2 stats interleaved -> h2T ----------
            h2T = persist.tile([128, 8, TC], bf16, tag="h2T")
            with tc.tile_pool(name="pj_ps", bufs=2, space="PSUM") as ps_pj, \
                 tc.tile_pool(name="ln2_work", bufs=4) as lnw2, \
                 tc.tile_pool(name="ln2_stats", bufs=2) as lns2, \
                 tc.tile_pool(name="ps_ln2", bufs=1, space="PSUM") as ps_ln2:
                mus2 = [ps_ln2.tile([1, 512], f32, tag=f"mu{c}",
                                    name=f"mu2_{c}") for c in range(2)]
                sqs2 = [ps_ln2.tile([1, 512], f32, tag=f"sq{c}",
                                    name=f"sq2_{c}") for c in range(2)]
                for o in range(8):
                    for c in range(2):
                        cs = bass.ds(c * 512, 512)
                        ps = ps_pj.tile([128, 512], f32, tag="pj",
                                        name=f"pj7_{o}_{c}")
                        nc.tensor.matmul(ps, wp_sb[:, NG - 1, o, :],
                                         attT[:, NG - 1, cs],
                                         start=True, stop=True)
                        nc.vector.tensor_tensor(out=x2T[:, o, cs], in0=ps,
                                                in1=x2T[:, o, cs], op=ALU.add)
                    sq = lnw2.tile([128, 1024], bf16, tag="sq")
                    if o % 2 == 0:
                        nc.scalar.activation(out=sq, in_=x2T[:, o, :],
                                             func=AF.Square)
                    else:
                        nc.vector.tensor_tensor(out=sq, in0=x2T[:, o, :],
                                                in1=x2T[:, o, :], op=ALU.mult)
                    for c in range(2):
                        cs = bass.ds(c * 512, 512)
                        nc.tensor.matmul(mus2[c], ones_f, x2T[:, o, cs],
                                         start=(o == 0), stop=(o == 7))
                        nc.tensor.matmul(sqs2[c], ones_r,
                                         sq[:, c * 512:(c + 1) * 512],
                                         start=(o == 0), stop=(o == 7))
                mu_b2, rs_b2 = ln_tail(mus2, sqs2, x2T, h2T, 0, 0, lnw2,
                                       lns2, ps_ln2, warm_pe=True)
                ln_apply(x2T, h2T, 0, mu_b2, rs_b2, lnw2)

            # free hT/hq/attT + attention work pools before FFN allocates uT
            hctx.close()

            # ---------- FFN (transposed throughout) ----------
            with tc.tile_pool(name="ffn_c", bufs=1) as fcc, \
                 tc.tile_pool(name="ffn_w", bufs=3) as fw, \
                 tc.tile_pool(name="ffn_x", bufs=2) as fx, \
                 tc.tile_pool(name="ps_u", bufs=2, space="PSUM") as ps_u, \
                 tc.tile_pool(name="ps_v", bufs=2, space="PSUM") as ps_v:
                b1t = fcc.tile([128, 32], f32)
                nc.sync.dma_start(out=b1t, in_=b1_d[:, :].rearrange("i p -> p i"))
                b2t = fcc.tile([128, 8], f32)
                nc.sync.dma_start(out=b2t, in_=b2_d[:, :].rearrange("o p -> p o"))
                uT = fcc.tile([128, 32, TC], bf16, tag="uT")
                # c-outer: FFN1 starts after only the c=0 half of the LN2
                # apply; w1 streams twice on the otherwise-idle Scalar queue
                for c in range(2):
                    cs = bass.ds(c * 512, 512)
                    for i in range(32):
                        w1t = fw.tile([128, 8, 128], bf16, tag="w1t")
                        nc.scalar.dma_start(
                            out=w1t, in_=w1_d[i].rearrange("k p c -> p k c"))
                        psu = ps_u.tile([128, 512], f32, tag="u")
                        for k in range(8):
                            nc.tensor.matmul(psu, w1t[:, k, :],
                                             h2T[:, k, cs],
                                             start=(k == 0), stop=(k == 7))
                        nc.scalar.activation(out=uT[:, i, cs], in_=psu,
                                             func=AF.Relu,
                                             bias=b1t[:, i:i + 1])
                for o in range(8):
                    w2t = fw.tile([128, 32, 128], bf16, tag="w2t")
                    nc.sync.dma_start(
                        out=w2t, in_=w2_d[o].rearrange("i p c -> p i c"))
                    pso = ps_v.tile([128, 2, 512], f32, tag="v")
                    for c in range(2):
                        cs = bass.ds(c * 512, 512)
                        for i in range(32):
                            nc.tensor.matmul(pso[:, c, :], w2t[:, i, :],
                                             uT[:, i, cs],
                                             start=(i == 0), stop=(i == 31))
                    ot = fx.tile([128, 1024], f32, tag="ot")
                    nc.vector.tensor_scalar_add(ot, pso, b2t[:, o:o + 1])
                    nc.vector.tensor_tensor(out=ot, in0=ot, in1=x2T[:, o, :],
                                            op=ALU.add)
                    nc.sync.dma_start(out=out_d[o * 128:(o + 1) * 128, :], in_=ot)

    nc.compile()
    return nc


def _prep_shared(wq, wk, wv, wp, bp, w1, b1, w2, b2, g1, be1, g2, be2):
    c = np.ascontiguousarray
    f = np.float32

    def cf(a):
        return c(np.asarray(a, f))

    g1 = cf(g1)
    be1 = cf(be1)
    g2 = cf(g2)
    be2 = cf(be2)
    wq0, wk0, wv0, w10 = cf(wq), cf(wk), cf(wv), cf(w1)
    # fold LN gains into weights; LN biases become post-projection biases
    wqs = wq0 * g1[:, None]
    wks = wk0 * g1[:, None]
    wvs = wv0 * g1[:, None]
    w1s = w10 * g2[:, None]
    qbias = (wq0.T @ be1).astype(f)
    kbias = (wk0.T @ be1).astype(f)
    vbias = (wv0.T @ be1).astype(f)
    b1n = (cf(b1) + w10.T @ be2).astype(f)
    return {
        "wqp": c(wqs.reshape(8, 128, 8, 128).transpose(2, 0, 1, 3)).astype(BF16),
        "wkp": c(wks.reshape(8, 128, 8, 128).transpose(2, 0, 1, 3)).astype(BF16),
        "wvp": c(wvs.reshape(8, 128, 8, 128).transpose(2, 0, 1, 3)).astype(BF16),
        "qbias": c(qbias.reshape(NG, 128)),
        "kbias": c(kbias.reshape(NG, 128)),
        "vbias": c(vbias.reshape(NG, 128)),
        "wpp": c(cf(wp).reshape(8, 128, 8, 128).transpose(0, 2, 1, 3)).astype(BF16),
        "w1p": c(w1s.reshape(8, 128, 32, 128).transpose(2, 0, 1, 3)).astype(BF16),
        "w2p": c(cf(w2).reshape(32, 128, 8, 128).transpose(2, 0, 1, 3)).astype(BF16),
        "b1t": c(b1n.reshape(32, 128)),
        "b2t": c(cf(b2).reshape(8, 128)),
        "bp": cf(bp),
    }


def _own_idx(p):
    return (np.arange(NJ)[:, None] * 1024 + p * QB + np.arange(QB)[None, :]).ravel()


def _masks(p):
    m = np.zeros((8, 128, QB), np.float32)
    k = np.arange(128)[:, None]
    q = np.arange(QB)[None, :]
    for rel in range(8):
        m[rel] = (128 * rel + k <= QB * p + q).astype(np.float32)
    return m


def _make_in_maps(x, shared):
    in_maps = []
    bp = shared["bp"]
    for cid in range(N_CORES):
        b, p = cid // 2, cid % 2
        xb = np.asarray(x[b], np.float32)
        idx = _own_idx(p)
        xq = xb[idx]
        m = {k: v for k, v in shared.items() if k != "bp"}
        m["xT"] = np.ascontiguousarray(xb.T).astype(BF16)
        m["xqT"] = np.ascontiguousarray(xq.T).astype(BF16)
        m["xoT"] = np.ascontiguousarray((xq + bp[None, :]).T.astype(np.float32))
        m["masks"] = _masks(p).astype(BF16)
        in_maps.append(m)
    return in_maps


def kernel(**inputs):
    from concourse.bass_utils import run_bass_kernel_spmd

    if "nc" not in _cache:
        _cache["nc"] = _build()
    nc = _cache["nc"]

    shared = _prep_shared(
        inputs["wq"], inputs["wk"], inputs["wv"], inputs["wp"], inputs["bp"],
        inputs["w1"], inputs["b1"], inputs["w2"], inputs["b2"],
        inputs["g1"], inputs["be1"], inputs["g2"], inputs["be2"])
    in_maps = _make_in_maps(inputs["x"], shared)

    res = run_bass_kernel_spmd(nc, in_maps, list(range(N_CORES)))
    out = np.empty((B, T, D), np.float32)
    for cid in range(N_CORES):
        b, p = cid // 2, cid % 2
        out[b][_own_idx(p)] = res.results[cid]["out"].T
    return out

